# revision 5
# baseline (speedup 1.0000x reference)
"""Trainium2 Bass kernel for nn_BBN_Layer (normalized cross-correlation
with a parts codebook). Batch-parallel over 8 NeuronCores, one image per
core.

Math (padding=0, valid conv, fs=32, H=W=256, P=64 parts):
The reference's 9 convolutions collapse (channel-uniform part_alpha
filters sum their input channels first) into ONE stacked 15-channel conv
with 128 output channels (64 numerator + 64 denominator):

  planes c0-2 : X1 = image*(1-fa)            weights W1 = rgb*pa
  plane  c3   : X2s = sum_c X1*bg            weights -pa
  planes c4-6 : X3 = ga^2                    weights W1^2
  planes c7-9 : X4 = 2*alpha_A*ga            weights W1
  plane  c10  : X5s = sum_c (ga*bg)^2        weights pa^2-2pa
  plane  c11  : X6s = sum_c 2*alpha_A*ga*bg  weights -pa
  planes c12-14: X7 = 2*ga^2*bg              weights W1*(1-pa)

  numer = conv_numer + sum(image*alpha_A) + sum(X2s)
  denom = conv_denom + sum(alpha_A^2) + sum(X5s) + sum(X6s)
  out   = numer / sqrt(I_norm * denom)

Conv-as-matmul: K = (channel, j2) = 15*8 = 120 partitions, M = 128
output channels, N = 450 (two output rows), accumulating 32 (filter row
i) x 4 (j1) fp32r matmuls per row-pair into one PSUM bank. The rhs is a
plain strided view into an 8-way shifted-replicated image window
(S[(c,j2), r, x] = X[c, r, x+j2]) built by a single overlapped-read DMA
from a DRAM plane buffer.
"""

import sys

sys.path.insert(0, "/opt/trn_rl_repo")

import numpy as np

import concourse.bass as bass
import concourse.mybir as mybir
from concourse import bacc, tile

f32 = mybir.dt.float32
f32r = mybir.dt.float32r
Alu = mybir.AluOpType
Act = mybir.ActivationFunctionType

H = W = 256
FS = 32
P = 64
HO = WO = H - FS + 1  # 225
NCH = 15  # stacked conv channels
NJ2 = 8  # shift replication factor
KP = NCH * NJ2  # 120 contraction partitions
NJ1 = FS // NJ2  # 4
NY = 22  # output rows per S window
NWIN_FULL = 10  # full windows cover rows 0..219; tail window covers 220..224


def _build_program():
    nc = bacc.Bacc()

    img_d = nc.declare_dram_parameter("img", [3, H * W], f32, isOutput=False)
    fa_d = nc.declare_dram_parameter("fa", [3, H * W], f32, isOutput=False)
    aA_d = nc.declare_dram_parameter("aA", [3, H * W], f32, isOutput=False)
    bg_d = nc.declare_dram_parameter("bg", [3, H * W], f32, isOutput=False)
    wpack_d = nc.declare_dram_parameter(
        "wpack", [KP, FS * NJ1 * 128], f32r, isOutput=False
    )
    out_d = nc.declare_dram_parameter("out", [P, HO, WO], f32, isOutput=True)

    with tile.TileContext(nc) as tc:
        with (
            tc.tile_pool(name="dram", bufs=1, space="DRAM") as dpool,
            tc.tile_pool(name="persist", bufs=1) as persist,
        ):
            # +1 dummy plane: the j2-overlapped S reads of the last window
            # run up to NJ2-1 elements past the last plane's end; the spill
            # lands in the dummy plane and is never consumed by a matmul.
            planes = dpool.tile([NCH + 1, H * W], f32r)
            wtile = persist.tile([KP, FS * NJ1 * 128], f32r)
            nc.sync.dma_start(wtile[:], wpack_d[:])
            bc = persist.tile([128, 4], f32)

            # ---------------- Phase A: plane prep + reductions --------------
            with (
                tc.tile_pool(name="prep", bufs=1) as prep,
                tc.tile_pool(name="ppsum", bufs=2, space="PSUM") as ppsum,
            ):
                ones128 = prep.tile([128, 1], f32)
                nc.vector.memset(ones128[:], 1.0)
                ones1 = prep.tile([1, 128], f32)
                nc.vector.memset(ones1[:], 1.0)

                # stats cols: 0-2 img*aA, 3 X2s, 4-6 aA^2, 7 X5s, 8 X6s,
                # 9-11 img^2
                stats = prep.tile([128, 12], f32)

                x2cs, x5cs, x6cs = [], [], []
                for c in range(3):
                    ic = prep.tile([128, 512], f32, tag=f"ic{c}")
                    fc = prep.tile([128, 512], f32, tag=f"fc{c}")
                    ac = prep.tile([128, 512], f32, tag=f"ac{c}")
                    gc = prep.tile([128, 512], f32, tag=f"gc{c}")
                    src = lambda d: d[c].rearrange("(p e) -> p e", p=128)
                    nc.sync.dma_start(ic[:], src(img_d))
                    nc.sync.dma_start(fc[:], src(fa_d))
                    nc.sync.dma_start(ac[:], src(aA_d))
                    nc.sync.dma_start(gc[:], src(bg_d))

                    ga = prep.tile([128, 512], f32, tag=f"ga{c}")
                    nc.vector.tensor_scalar(ga[:], fc[:], -1.0, 1.0, Alu.mult, Alu.add)

                    x1 = prep.tile([128, 512], f32r, tag=f"x1{c}")
                    nc.vector.tensor_tensor(x1[:], ic[:], ga[:], Alu.mult)
                    x2c = prep.tile([128, 512], f32, tag=f"x2{c}")
                    nc.vector.tensor_tensor(
                        x2c[:], x1[:].bitcast(f32), gc[:], Alu.mult
                    )
                    x2cs.append(x2c)
                    x3 = prep.tile([128, 512], f32r, tag=f"x3{c}")
                    nc.vector.tensor_tensor(x3[:], ga[:], ga[:], Alu.mult)
                    t4 = prep.tile([128, 512], f32, tag=f"t4{c}")
                    nc.vector.tensor_tensor(t4[:], ac[:], ga[:], Alu.mult)
                    x4 = prep.tile([128, 512], f32r, tag=f"x4{c}")
                    nc.vector.tensor_tensor(x4[:], t4[:], t4[:], Alu.add)
                    gb = prep.tile([128, 512], f32, tag=f"gb{c}")
                    nc.vector.tensor_tensor(gb[:], ga[:], gc[:], Alu.mult)
                    x5c = prep.tile([128, 512], f32, tag=f"x5{c}")
                    nc.vector.tensor_tensor(x5c[:], gb[:], gb[:], Alu.mult)
                    x5cs.append(x5c)
                    x6c = prep.tile([128, 512], f32, tag=f"x6{c}")
                    nc.vector.tensor_tensor(
                        x6c[:], x4[:].bitcast(f32), gc[:], Alu.mult
                    )
                    x6cs.append(x6c)
                    t7 = prep.tile([128, 512], f32, tag=f"t7{c}")
                    nc.vector.tensor_tensor(t7[:], x3[:].bitcast(f32), gc[:], Alu.mult)
                    x7 = prep.tile([128, 512], f32r, tag=f"x7{c}")
                    nc.vector.tensor_tensor(x7[:], t7[:], t7[:], Alu.add)

                    # reductions
                    tr = prep.tile([128, 512], f32, tag=f"tr{c}")
                    nc.vector.tensor_tensor(tr[:], ic[:], ac[:], Alu.mult)
                    nc.vector.tensor_reduce(
                        stats[:, c : c + 1], tr[:], mybir.AxisListType.X, Alu.add
                    )
                    tr2 = prep.tile([128, 512], f32, tag=f"tr2{c}")
                    nc.vector.tensor_tensor(tr2[:], ac[:], ac[:], Alu.mult)
                    nc.vector.tensor_reduce(
                        stats[:, 4 + c : 5 + c], tr2[:], mybir.AxisListType.X, Alu.add
                    )
                    tr3 = prep.tile([128, 512], f32, tag=f"tr3{c}")
                    nc.vector.tensor_tensor(tr3[:], ic[:], ic[:], Alu.mult)
                    nc.vector.tensor_reduce(
                        stats[:, 9 + c : 10 + c], tr3[:], mybir.AxisListType.X, Alu.add
                    )

                    # plane DMAs (c0-2: X1, c4-6: X3, c7-9: X4, c12-14: X7)
                    dst = lambda ch: planes[ch].rearrange("(p e) -> p e", p=128)
                    nc.sync.dma_start(dst(c), x1[:])
                    nc.sync.dma_start(dst(4 + c), x3[:])
                    nc.sync.dma_start(dst(7 + c), x4[:])
                    nc.sync.dma_start(dst(12 + c), x7[:])

                # channel sums -> f32r planes + their reductions
                for ch, tiles_, col in ((3, x2cs, 3), (10, x5cs, 7), (11, x6cs, 8)):
                    tsum = prep.tile([128, 512], f32, tag=f"tsum{ch}")
                    nc.vector.tensor_tensor(
                        tsum[:], tiles_[0][:], tiles_[1][:], Alu.add
                    )
                    xs = prep.tile([128, 512], f32r, tag=f"xs{ch}")
                    nc.vector.tensor_tensor(xs[:], tsum[:], tiles_[2][:], Alu.add)
                    nc.vector.tensor_reduce(
                        stats[:, col : col + 1],
                        xs[:].bitcast(f32),
                        mybir.AxisListType.X,
                        Alu.add,
                    )
                    nc.sync.dma_start(
                        planes[ch].rearrange("(p e) -> p e", p=128), xs[:]
                    )

                # cross-partition reduce -> per-image scalars
                pstat = ppsum.tile([1, 12], f32)
                nc.tensor.matmul(pstat[:], ones128[:], stats[:], start=True, stop=True)
                sc = prep.tile([1, 4], f32)
                # sc: 0=ns, 1=I_norm, 2=I_norm*ds, 3=ds
                nc.vector.tensor_reduce(
                    sc[:, 0:1], pstat[:, 0:4], mybir.AxisListType.X, Alu.add
                )
                nc.vector.tensor_reduce(
                    sc[:, 3:4], pstat[:, 4:9], mybir.AxisListType.X, Alu.add
                )
                nc.vector.tensor_reduce(
                    sc[:, 1:2], pstat[:, 9:12], mybir.AxisListType.X, Alu.add
                )
                nc.vector.tensor_tensor(sc[:, 2:3], sc[:, 1:2], sc[:, 3:4], Alu.mult)
                pbc = ppsum.tile([128, 4], f32)
                nc.tensor.matmul(pbc[:], ones1[:], sc[:], start=True, stop=True)
                nc.vector.tensor_copy(bc[:], pbc[:])

            # ---------------- Phase B: conv ----------------------------------
            with (
                tc.tile_pool(name="spool", bufs=2) as spool,
                tc.tile_pool(name="cpsum", bufs=8, space="PSUM") as cpsum,
                tc.tile_pool(name="evac", bufs=3) as evac,
            ):
                ph = planes[:].tensor
                poff = planes[:].offset

                # fp32r matmuls need an even innermost moving count; compute
                # WO+1=226 columns and drop the garbage last column at the
                # output DMA.
                WE = WO + 1

                def do_pair(stile, y0, yloc, nrows):
                    """Output rows y0+yloc .. y0+yloc+nrows-1 (nrows in 1,2)."""
                    pt = cpsum.tile([128, nrows, WE], f32, tag="pt")
                    for i in range(FS):
                        for j1 in range(NJ1):
                            g = i * NJ1 + j1
                            nc.tensor.matmul(
                                pt[:],
                                wtile[:, g * 128 : (g + 1) * 128],
                                stile[:, yloc + i : yloc + i + nrows,
                                      j1 * NJ2 : j1 * NJ2 + WE],
                                start=(g == 0),
                                stop=(g == FS * NJ1 - 1),
                            )
                    sq = evac.tile([128, nrows, WE], f32, tag="sq")
                    nc.scalar.activation(
                        sq[64:128], pt[64:128], Act.Sqrt,
                        bias=bc[64:128, 2:3], scale=bc[64:128, 1:2],
                    )
                    rec = evac.tile([128, nrows, WE], f32, tag="rec")
                    nc.vector.reciprocal(rec[64:128], sq[64:128])
                    rec2 = evac.tile([64, nrows, WE], f32, tag="rec2")
                    nc.sync.dma_start(rec2[:], rec[64:128])
                    num = evac.tile([64, nrows, WE], f32, tag="num")
                    nc.vector.tensor_scalar(
                        num[:], pt[0:64], bc[0:64, 0:1], None, Alu.add
                    )
                    res = evac.tile([64, nrows, WE], f32, tag="res")
                    nc.vector.tensor_tensor(res[:], num[:], rec2[:], Alu.mult)
                    y = y0 + yloc
                    nc.sync.dma_start(out_d[:, y : y + nrows, :], res[:, :, 0:WO])

                for w in range(NWIN_FULL + 1):
                    y0 = w * NY
                    ny = NY if w < NWIN_FULL else HO - NWIN_FULL * NY  # 22 or 5
                    rl = min(ny + FS - 1, H - y0)
                    stile = spool.tile([KP, rl, W], f32r, tag="stile")
                    nc.sync.dma_start(
                        stile[:],
                        bass.AP(ph, poff + y0 * W, [[H * W, NCH], [1, NJ2], [1, rl * W]]),
                    )
                    k = 0
                    while k + 2 <= ny:
                        do_pair(stile, y0, k, 2)
                        k += 2
                    if k < ny:
                        do_pair(stile, y0, k, 1)

    nc.compile()
    return nc


def _pack_weights(parts: np.ndarray) -> np.ndarray:
    parts = parts.astype(np.float32)
    rgb = parts[:, :3]  # [64,3,32,32]
    pa = parts[:, 3:4]  # [64,1,32,32]
    w1 = rgb * pa
    wstack = np.zeros((128, NCH, FS, FS), np.float32)
    wstack[:P, 0:3] = w1
    wstack[:P, 3] = -pa[:, 0]
    wstack[P:, 4:7] = w1 * w1
    wstack[P:, 7:10] = w1
    wstack[P:, 10] = pa[:, 0] * pa[:, 0] - 2.0 * pa[:, 0]
    wstack[P:, 11] = -pa[:, 0]
    wstack[P:, 12:15] = w1 * (1.0 - pa)
    # [m, c, i, j1, j2] -> [c, j2, i, j1, m]
    wp = wstack.reshape(128, NCH, FS, NJ1, NJ2).transpose(1, 4, 2, 3, 0)
    return np.ascontiguousarray(wp).reshape(KP, FS * NJ1 * 128)


_CACHE = {}


def _get_runner():
    """Build the program once and keep a reusable jitted executor."""
    if "run" in _CACHE:
        return _CACHE["run"]

    import jax
    from jax.sharding import Mesh, PartitionSpec
    from jax.experimental.shard_map import shard_map
    from concourse import bass2jax
    from concourse.bass2jax import _bass_exec_p, install_neuronx_cc_hook

    nc = _build_program()
    install_neuronx_cc_hook()

    partition_name = (
        nc.partition_id_tensor.name if nc.partition_id_tensor else None
    )
    in_names, out_names, out_avals = [], [], []
    for alloc in nc.m.functions[0].allocations:
        if not isinstance(alloc, mybir.MemoryLocationSet):
            continue
        name = alloc.memorylocations[0].name
        if alloc.kind == "ExternalInput":
            if name != partition_name:
                in_names.append(name)
        elif alloc.kind == "ExternalOutput":
            out_names.append(name)
            out_avals.append(
                jax.core.ShapedArray(
                    tuple(alloc.tensor_shape), mybir.dt.np(alloc.dtype)
                )
            )
    n_params = len(in_names)
    n_outs = len(out_names)
    all_names = in_names + out_names
    if partition_name is not None:
        all_names = all_names + [partition_name]

    def _body(*args):
        operands = list(args)
        if partition_name is not None:
            operands.append(bass2jax.partition_id_tensor())
        return tuple(
            _bass_exec_p.bind(
                *operands,
                out_avals=tuple(out_avals),
                in_names=tuple(all_names),
                out_names=tuple(out_names),
                lowering_input_output_aliases=(),
                sim_require_finite=True,
                sim_require_nnan=True,
                nc=nc,
            )
        )

    n_cores = 8
    devices = jax.devices()[:n_cores]
    mesh = Mesh(np.asarray(devices), ("core",))
    donate = tuple(range(n_params, n_params + n_outs))
    sharded = jax.jit(
        shard_map(
            _body,
            mesh=mesh,
            in_specs=(PartitionSpec("core"),) * (n_params + n_outs),
            out_specs=(PartitionSpec("core"),) * n_outs,
            check_rep=False,
        ),
        donate_argnums=donate,
        keep_unused=True,
    )

    def run(in_maps):
        per_core = [[np.asarray(m[n]) for n in in_names] for m in in_maps]
        concat_in = [
            np.concatenate([per_core[c][i] for c in range(n_cores)], axis=0)
            for i in range(n_params)
        ]
        zero_outs = [
            np.zeros((av.shape[0] * n_cores,) + av.shape[1:], av.dtype)
            for av in out_avals
        ]
        outs = sharded(*concat_in, *zero_outs)
        outs = [np.asarray(o) for o in outs]
        return [
            {
                name: np.split(outs[i], n_cores, axis=0)[c]
                for i, name in enumerate(out_names)
            }
            for c in range(n_cores)
        ]

    _CACHE["run"] = run
    return run


def kernel(image, parts, foreground_alpha, alpha_A, background, padding=0):
    run = _get_runner()
    wpack = _pack_weights(parts)
    B = image.shape[0]
    in_maps = [
        {
            "img": np.ascontiguousarray(image[b], np.float32).reshape(3, H * W),
            "fa": np.ascontiguousarray(
                foreground_alpha[b], np.float32
            ).reshape(3, H * W),
            "aA": np.ascontiguousarray(alpha_A[b], np.float32).reshape(3, H * W),
            "bg": np.ascontiguousarray(background[b], np.float32).reshape(3, H * W),
            "wpack": wpack,
        }
        for b in range(B)
    ]
    results = run(in_maps)
    return np.stack([results[b]["out"] for b in range(B)], axis=0)
